# revision 1
# baseline (speedup 1.0000x reference)
"""Trainium2 Bass kernel for nn_LookaheadModel (topk_masking).

Sharding: data-parallel over batch B=8 (one batch element per core) for the
encoder; tiny AllGather of per-batch context vectors; vocab-sharded output
projection (each core computes logits[:, shard]).

Layout: activations kept feature-major (D on partitions, T on free dim) so all
matmuls contract over features. Top-256 gate selection is done exactly via a
parallel threshold search + tie-fix, then the read-head is a masked softmax and
a weighted reduction over all T (no gather of selected rows needed).

Self-contained: only needs numpy + the system-installed concourse package.
"""

import numpy as np

import bass_rust
import concourse.bass as bass
import concourse.mybir as mybir
from concourse.bass_utils import run_bass_kernel_spmd
from concourse.tile import TileContext

AF = mybir.ActivationFunctionType
ALU = mybir.AluOpType
F32 = mybir.dt.float32
F32R = mybir.dt.float32r
F16 = mybir.dt.float16
I32 = mybir.dt.int32

# ---------------------------------------------------------------------------
# Workaround: this walrus build rejects any instruction carrying more than one
# sync-wait command. Hoist excess waits onto same-engine NOPs (sequential on
# the same engine queue, so semantically identical).
# ---------------------------------------------------------------------------
_MAX_WAITS = 1
_nop_counter = [0]


def _split_waits_in_ordered(nc, ordered):
    for bb_name, insts in ordered.items():
        out = []
        for inst in insts:
            si = inst.sync_info
            waits = list(si.on_wait) if si and si.on_wait else []
            if len(waits) > _MAX_WAITS:
                spill, keep = waits[:-_MAX_WAITS], waits[-_MAX_WAITS:]
                for i in range(0, len(spill), _MAX_WAITS):
                    _nop_counter[0] += 1
                    nop = bass_rust.InstNoOp(name=f"WSPILL-{_nop_counter[0]}")
                    nop.engine = inst.engine
                    nop.sync_info = mybir.SyncInfo(
                        on_wait=list(spill[i : i + _MAX_WAITS]), on_update=[]
                    )
                    nop.bass_nofuse = True
                    nc.register_instruction(nop, overwrite=True)
                    out.append(nop)
                si.on_wait = keep
            out.append(inst)
        if len(out) != len(insts):
            insts[:] = out


_orig_lower = TileContext._lower_ordered_insts
_orig_drain = TileContext._drain_and_barrier


def _lower_with_split(self, ordered):
    _split_waits_in_ordered(self.nc, ordered)
    return _orig_lower(self, ordered)


def _drain_and_barrier_split(self, tick_clock, wait_clock):
    nc = self.nc
    sc = bass_rust.ScopedClock({None: tick_clock.global_clock})
    drain_inst = nc.sync.drain()
    wait_clock.add_sem_waits(drain_inst.ins, sc)
    si = drain_inst.ins.sync_info
    waits = list(si.on_wait or [])
    if len(waits) > _MAX_WAITS:
        si.on_wait = waits[:_MAX_WAITS]
        rest = waits[_MAX_WAITS:]
        for i in range(0, len(rest), _MAX_WAITS):
            nop = nc.sync.nop(nofuse=True, hint=f"drain_wait_spill_{i}")
            nop.ins.sync_info = mybir.SyncInfo(
                on_wait=list(rest[i : i + _MAX_WAITS]), on_update=[]
            )
    nc.all_engine_barrier()
    popped = nc._tile_sem_poison_stack.pop()
    assert popped is self._sem_poison
    nc.clear_and_free_semaphores(list(self.sems.allocated().values()))
    nc.all_engine_barrier()


def _apply_patch():
    TileContext._drain_and_barrier = _drain_and_barrier_split
    TileContext._lower_ordered_insts = _lower_with_split


# ---------------------------------------------------------------------------
# Problem constants
# ---------------------------------------------------------------------------
V, D, SLOTS, K = 50257, 512, 256, 8
B, T = 8, 4096
NCORES = 8
VS = 6283  # vocab shard width per core; 8*6283 = 50264 >= V
NCH = 8  # T chunks of width 512
CW = 512
NK = D // 128  # 4 feature tiles
NF = 2 * D // 128  # 8 hidden tiles
BIG = 1.0e30
EPS = 1e-5

# When False, all heavy matmuls run in float32r (tf32-like, ~1e-4 rel error,
# ~3x faster). When True, everything is exact fp32.
PURE_FP32 = False
DEBUG_HT = False  # adds a (D, T) dump of hT per core (bring-up only)

_cache = {}


def _dt():
    return F32 if PURE_FP32 else F32R


def build_bass():
    _apply_patch()
    DT = _dt()
    nc = bass.Bass(trn_type="TRN2", num_devices=NCORES)

    # ---- I/O ----
    embed = nc.dram_tensor("embed", (V, D), F32, kind="ExternalInput")
    seq_idx = nc.dram_tensor("seq_idx", (128, 32), I32, kind="ExternalInput")
    w1 = nc.dram_tensor("w1", (D, 2 * D), F32, kind="ExternalInput")
    w2 = nc.dram_tensor("w2", (2 * D, D), F32, kind="ExternalInput")
    qw = nc.dram_tensor("qw", (D, D), F32, kind="ExternalInput")
    b1c = nc.dram_tensor("b1c", (128, NF), F32, kind="ExternalInput")
    b2c = nc.dram_tensor("b2c", (128, NK), F32, kind="ExternalInput")
    lngr = nc.dram_tensor("lngr", (1, D), F32, kind="ExternalInput")
    lnbc = nc.dram_tensor("lnbc", (128, NK), F32, kind="ExternalInput")
    gwc = nc.dram_tensor("gwc", (128, 2 * NK), F32, kind="ExternalInput")
    qbc = nc.dram_tensor("qbc", (128, NK), F32, kind="ExternalInput")
    FDT = F32 if PURE_FP32 else F16  # final projection dtype
    wout = nc.dram_tensor("wout", (D, VS), FDT, kind="ExternalInput")
    bout = nc.dram_tensor("bout", (1, VS), F32, kind="ExternalInput")
    ident_in = nc.dram_tensor("ident", (128, 128), F32, kind="ExternalInput")
    onesc_in = nc.dram_tensor("onesc", (128, 1), F32, kind="ExternalInput")
    ones1x128_in = nc.dram_tensor("ones1x128", (1, 128), F32, kind="ExternalInput")
    ones1x8_in = nc.dram_tensor("ones1x8", (1, 8), F32, kind="ExternalInput")
    alpha_in = nc.dram_tensor("alphac", (128, 1), F32, kind="ExternalInput")
    cntrec_in = nc.dram_tensor("cntrec", (1, T), F32, kind="ExternalInput")
    sc_in = nc.dram_tensor("sc_in", (1, 4), F32, kind="ExternalInput")  # eps, gate_b

    logits = nc.dram_tensor("logits", (B, VS), F32, kind="ExternalOutput")
    dbg = nc.dram_tensor("dbg", (5, T), F32, kind="ExternalOutput")
    if DEBUG_HT:
        htdump = nc.dram_tensor("htdump", (D, T), F32, kind="ExternalOutput")

    cc_in = nc.dram_tensor("cc_in", (128, NK), F32, kind="Internal")
    cc_out = nc.dram_tensor(
        "cc_out", (128 * NCORES, NK), F32, kind="Internal", addr_space="Shared"
    )
    zrow_d = nc.dram_tensor("zrow_d", (1, T), F32, kind="Internal")

    with TileContext(nc) as tc:
        with tc.tile_pool(name="consts", bufs=1) as cpool:
            # ---------------- persistent constants ----------------
            ident = cpool.tile([128, 128], F32, name="ident_t")
            nc.sync.dma_start(ident[:], ident_in[:])
            b1t = cpool.tile([128, NF], F32, name="b1t")
            nc.sync.dma_start(b1t[:], b1c[:])
            b2t = cpool.tile([128, NK], F32, name="b2t")
            nc.sync.dma_start(b2t[:], b2c[:])
            lngf = cpool.tile([1, D], F32, name="lngf")
            nc.sync.dma_start(lngf[:], lngr[:])
            lngt = cpool.tile([1, D], DT, name="lngt")
            nc.vector.tensor_copy(lngt[:], lngf[:])
            lnbt = cpool.tile([128, NK], F32, name="lnbt")
            nc.sync.dma_start(lnbt[:], lnbc[:])
            gwf32 = cpool.tile([128, 2 * NK], F32, name="gwf32")
            nc.sync.dma_start(gwf32[:], gwc[:])
            gwt = cpool.tile([128, 2 * NK], DT, name="gwt")
            nc.vector.tensor_copy(gwt[:], gwf32[:])
            qbt = cpool.tile([128, NK], F32, name="qbt")
            nc.sync.dma_start(qbt[:], qbc[:])
            onescol = cpool.tile([128, 1], F32, name="onescol")
            nc.sync.dma_start(onescol[:], onesc_in[:])
            onescol_r = cpool.tile([128, 1], DT, name="onescol_r")
            nc.vector.tensor_copy(onescol_r[:], onescol[:])
            ones1x128 = cpool.tile([1, 128], F32, name="ones1x128")
            nc.sync.dma_start(ones1x128[:], ones1x128_in[:])
            ones1x8 = cpool.tile([1, 8], F32, name="ones1x8")
            nc.sync.dma_start(ones1x8[:], ones1x8_in[:])
            alphac = cpool.tile([128, 1], F32, name="alphac_t")
            nc.sync.dma_start(alphac[:], alpha_in[:])
            scin = cpool.tile([1, 4], F32, name="scin")
            nc.sync.dma_start(scin[:], sc_in[:])
            eps_ap = scin[0:1, 0:1]
            gb_ap = scin[0:1, 1:2]
            sidx = cpool.tile([128, 32], I32, name="sidx")
            nc.sync.dma_start(sidx[:], seq_idx[:])
            strip = cpool.tile([1, 64], F32, name="strip")
            qr = cpool.tile([128, NK], DT, name="qr")
            ctx4 = cpool.tile([128, NK], F32, name="ctx4")
            ctxall = cpool.tile([128, 32], F32, name="ctxall")

            with tc.tile_pool(name="hT", bufs=1) as hpool:
                hT = [hpool.tile([128, T], DT, name=f"hT{k}") for k in range(NK)]

                # ---------------- phase A: gather + FFN + LN ----------------
                with (
                    tc.tile_pool(name="wts", bufs=1) as wpool,
                    tc.tile_pool(name="gat", bufs=3) as gpool,
                    tc.tile_pool(name="x0p", bufs=2) as x0pool,
                    tc.tile_pool(name="ap", bufs=1) as apool,
                    tc.tile_pool(name="yp", bufs=2) as ypool,
                    tc.tile_pool(name="sqp", bufs=1) as sqpool,
                    tc.tile_pool(name="t12", bufs=2) as tpool,
                    tc.tile_pool(name="stats", bufs=1) as spool,
                    tc.tile_pool(name="pstp", bufs=1, space="PSUM") as pstp,
                    tc.tile_pool(name="psa", bufs=1, space="PSUM") as psa,
                    tc.tile_pool(name="psf", bufs=1, space="PSUM") as psf,
                    tc.tile_pool(name="pss", bufs=1, space="PSUM") as pss,
                    tc.tile_pool(name="psr", bufs=1, space="PSUM") as psr,
                ):
                    # weights (rounded to DT once)
                    w1r, w2r, qwr = [], [], []
                    for k in range(NK):
                        wf = wpool.tile([128, 2 * D], F32, name=f"w1f{k}", tag="w1f")
                        nc.sync.dma_start(wf[:], w1[128 * k : 128 * (k + 1), :])
                        wr = wpool.tile([128, 2 * D], DT, name=f"w1r{k}")
                        nc.vector.tensor_copy(wr[:], wf[:])
                        w1r.append(wr)
                    for k in range(NF):
                        wf = wpool.tile([128, D], F32, name=f"w2f{k}", tag="w2f")
                        nc.sync.dma_start(wf[:], w2[128 * k : 128 * (k + 1), :])
                        wr = wpool.tile([128, D], DT, name=f"w2r{k}")
                        nc.vector.tensor_copy(wr[:], wf[:])
                        w2r.append(wr)
                    for k in range(NK):
                        wf = wpool.tile([128, D], F32, name=f"qwf{k}", tag="qwf")
                        nc.sync.dma_start(wf[:], qw[128 * k : 128 * (k + 1), :])
                        wr = wpool.tile([128, D], DT, name=f"qwr{k}")
                        nc.vector.tensor_copy(wr[:], wf[:])
                        qwr.append(wr)

                    for ch in range(NCH):
                        sl = slice(ch * CW, (ch + 1) * CW)
                        x0 = [
                            x0pool.tile([128, CW], DT, name=f"x0_{k}", tag=f"x0_{k}")
                            for k in range(NK)
                        ]
                        for blk in range(4):
                            tb = 4 * ch + blk
                            g = gpool.tile([128, D], F32, name="g", tag="g")
                            nc.gpsimd.indirect_dma_start(
                                out=g[:],
                                out_offset=None,
                                in_=embed[:],
                                in_offset=bass.IndirectOffsetOnAxis(
                                    ap=sidx[:, tb : tb + 1], axis=0
                                ),
                            )
                            tp = pstp.tile([128, D], F32, tag="tp")
                            for k in range(NK):
                                nc.tensor.transpose(
                                    tp[:, 128 * k : 128 * (k + 1)],
                                    g[:, 128 * k : 128 * (k + 1)],
                                    ident[:],
                                )
                            for k in range(NK):
                                if k % 2 == 0:
                                    nc.vector.tensor_copy(
                                        x0[k][:, 128 * blk : 128 * (blk + 1)],
                                        tp[:, 128 * k : 128 * (k + 1)],
                                    )
                                else:
                                    nc.scalar.activation(
                                        x0[k][:, 128 * blk : 128 * (blk + 1)],
                                        tp[:, 128 * k : 128 * (k + 1)],
                                        AF.Copy,
                                    )
                        # layer 1 + relu
                        af = [
                            apool.tile([128, CW], DT, name=f"af{m}", tag=f"af{m}")
                            for m in range(NF)
                        ]
                        for m in range(NF):
                            ps = psa.tile([128, CW], F32, tag="psa")
                            for k in range(NK):
                                nc.tensor.matmul(
                                    ps[:],
                                    lhsT=w1r[k][:, 128 * m : 128 * (m + 1)],
                                    rhs=x0[k][:],
                                    start=(k == 0),
                                    stop=(k == NK - 1),
                                )
                            nc.scalar.activation(
                                af[m][:], ps[:], AF.Relu, bias=b1t[:, m : m + 1]
                            )
                        # layer 2 + bias + residual -> y
                        yc = [
                            ypool.tile([128, CW], DT, name=f"yc{m}", tag=f"yc{m}")
                            for m in range(NK)
                        ]
                        for m in range(NK):
                            ps = psf.tile([128, CW], F32, tag="psf")
                            for k in range(NF):
                                nc.tensor.matmul(
                                    ps[:],
                                    lhsT=w2r[k][:, 128 * m : 128 * (m + 1)],
                                    rhs=af[k][:],
                                    start=(k == 0),
                                    stop=(k == NF - 1),
                                )
                            nc.vector.scalar_tensor_tensor(
                                out=yc[m][:],
                                in0=ps[:],
                                scalar=b2t[:, m : m + 1],
                                in1=x0[m][:].bitcast(F32),
                                op0=ALU.add,
                                op1=ALU.add,
                            )
                        # LN stats
                        ps1 = pss.tile([1, CW], F32, tag="ps1")
                        ps2 = pss.tile([1, CW], F32, tag="ps2")
                        for m in range(NK):
                            nc.tensor.matmul(
                                ps1[:],
                                lhsT=onescol_r[:],
                                rhs=yc[m][:],
                                start=(m == 0),
                                stop=(m == NK - 1),
                            )
                        sq = sqpool.tile([128, CW], DT, name="sq", tag="sq")
                        for m in range(NK):
                            nc.scalar.activation(sq[:], yc[m][:], AF.Square)
                            nc.tensor.matmul(
                                ps2[:],
                                lhsT=onescol_r[:],
                                rhs=sq[:],
                                start=(m == 0),
                                stop=(m == NK - 1),
                            )
                        # stats rows: m, ex2, tmp, r, mr (free-packed, partition 0)
                        st = spool.tile([1, 5 * CW], F32, name="st", tag="st")
                        m_r = st[0:1, 0:CW]
                        ex2_r = st[0:1, CW : 2 * CW]
                        tmp_r = st[0:1, 2 * CW : 3 * CW]
                        r_r = st[0:1, 3 * CW : 4 * CW]
                        mr_r = st[0:1, 4 * CW : 5 * CW]
                        nc.vector.tensor_scalar(
                            out=m_r, in0=ps1[:], scalar1=1.0 / D, scalar2=None,
                            op0=ALU.mult,
                        )
                        nc.vector.tensor_scalar(
                            out=ex2_r, in0=ps2[:], scalar1=1.0 / D, scalar2=None,
                            op0=ALU.mult,
                        )
                        nc.vector.tensor_mul(tmp_r, m_r, m_r)
                        nc.vector.tensor_sub(r_r, ex2_r, tmp_r)  # var -> r slot
                        nc.scalar.activation(tmp_r, r_r, AF.Ln, bias=eps_ap)
                        nc.scalar.activation(r_r, tmp_r, AF.Exp, scale=-0.5)
                        nc.vector.tensor_mul(mr_r, m_r, r_r)
                        strr = spool.tile([1, 2 * CW], DT, name="strr", tag="strr")
                        r_rr = strr[0:1, 0:CW]
                        nc.vector.tensor_copy(r_rr, r_r)
                        mr_rr = strr[0:1, CW : 2 * CW]
                        nc.vector.tensor_copy(mr_rr, mr_r)
                        # apply: h = y*(g*r)B - (g*m*r)B + b, the (128,CW) factors
                        # built by K=1 matmuls with lhsT = g-row slice
                        for m in range(NK):
                            gsl = lngt[0:1, 128 * m : 128 * (m + 1)]
                            psRG = psr.tile([128, CW], F32, tag="psRG")
                            nc.tensor.matmul(
                                psRG[:], lhsT=gsl, rhs=r_rr, start=True, stop=True
                            )
                            psMRG = psr.tile([128, CW], F32, tag="psMRG")
                            nc.tensor.matmul(
                                psMRG[:], lhsT=gsl, rhs=mr_rr, start=True, stop=True
                            )
                            t1 = tpool.tile([128, CW], F32, name="t1", tag="t1")
                            nc.vector.tensor_mul(t1[:], yc[m][:].bitcast(F32), psRG[:])
                            nc.vector.scalar_tensor_tensor(
                                out=hT[m][:, sl],
                                in0=t1[:],
                                scalar=lnbt[:, m : m + 1],
                                in1=psMRG[:],
                                op0=ALU.add,
                                op1=ALU.subtract,
                            )

                    # q vector (from last token)
                    with tc.tile_pool(name="psq", bufs=1, space="PSUM") as psq:
                        for j in range(NK):
                            pq = psq.tile([128, 1], F32, tag="pq")
                            for k in range(NK):
                                nc.tensor.matmul(
                                    pq[:],
                                    lhsT=qwr[k][:, 128 * j : 128 * (j + 1)].bitcast(F32),
                                    rhs=hT[k][:, T - 1 : T].bitcast(F32),
                                    start=(k == 0),
                                    stop=(k == NK - 1),
                                )
                            nc.vector.tensor_add(
                                qr[:, j : j + 1], pq[:], qbt[:, j : j + 1]
                            )

                # ---------------- phase B: scores + gate ----------------
                with tc.tile_pool(name="arena", bufs=1) as arena_pool:
                    arena = arena_pool.tile([1, 5 * T], F32, name="arena")
                    s_cnt = arena[0:1, 0:T]  # cntrec -> maskeq -> sel -> e
                    s_z = arena[0:1, T : 2 * T]  # z_pre -> incl -> maskgtv -> selm1
                    s_phi = arena[0:1, 2 * T : 3 * T]  # phi -> u -> z -> masked
                    s_cs = arena[0:1, 3 * T : 4 * T]  # csphi -> ucnt -> fill
                    s_sc = arena[0:1, 4 * T : 5 * T]  # scores s (computed early)
                    nc.sync.dma_start(s_cnt, cntrec_in[:])

                    with tc.tile_pool(name="pssc", bufs=2, space="PSUM") as psc:
                        for ch in range(NCH):
                            sl = slice(ch * CW, (ch + 1) * CW)
                            pssc = psc.tile([1, CW], F32, tag="pssc")
                            for k in range(NK):
                                nc.tensor.matmul(
                                    pssc[:],
                                    lhsT=qr[:, k : k + 1],
                                    rhs=hT[k][:, sl],
                                    start=(k == 0),
                                    stop=(k == NK - 1),
                                )
                            nc.vector.tensor_copy(s_sc[0:1, sl], pssc[:])
                    nc.sync.dma_start(dbg[2:3, :], s_sc)

                    with tc.tile_pool(name="psg", bufs=2, space="PSUM") as psg:
                        for ch in range(NCH):
                            sl = slice(ch * CW, (ch + 1) * CW)
                            pzh = psg.tile([1, CW], F32, tag="pzh")
                            pph = psg.tile([1, CW], F32, tag="pph")
                            for k in range(NK):
                                nc.tensor.matmul(
                                    pzh[:],
                                    lhsT=gwt[:, k : k + 1],
                                    rhs=hT[k][:, sl],
                                    start=(k == 0),
                                    stop=(k == NK - 1),
                                )
                                nc.tensor.matmul(
                                    pph[:],
                                    lhsT=gwt[:, NK + k : NK + k + 1],
                                    rhs=hT[k][:, sl],
                                    start=(k == 0),
                                    stop=(k == NK - 1),
                                )
                            nc.vector.tensor_copy(s_z[0:1, sl], pzh[:])
                            nc.vector.tensor_copy(s_phi[0:1, sl], pph[:])
                    # cs = cumsum(phi); u = windowed sum; z = z_pre + gb + u*cntrec
                    nc.vector.tensor_tensor_scan(
                        s_cs, s_phi, s_phi, 0.0, op0=ALU.add, op1=ALU.bypass
                    )
                    nc.vector.tensor_sub(
                        s_phi[0:1, 0 : T - K], s_cs[0:1, K:T], s_cs[0:1, 0 : T - K]
                    )
                    nc.vector.tensor_scalar(
                        out=s_phi[0:1, T - K : T],
                        in0=s_cs[0:1, T - K : T],
                        scalar1=s_cs[0:1, T - 1 : T],
                        scalar2=-1.0,
                        op0=ALU.subtract,
                        op1=ALU.mult,
                    )
                    nc.vector.tensor_mul(s_cs, s_phi, s_cnt)  # ucnt
                    nc.vector.scalar_tensor_tensor(
                        out=s_phi, in0=s_z, scalar=gb_ap, in1=s_cs,
                        op0=ALU.add, op1=ALU.add,
                    )  # z
                    z_row = s_phi
                    nc.sync.dma_start(dbg[0:1, :], z_row)

                    # ---------------- selection ----------------
                    with (
                        tc.tile_pool(name="tail", bufs=1) as tailp,
                        tc.tile_pool(name="pssm", bufs=1, space="PSUM") as ps_small,
                        tc.tile_pool(name="pswd", bufs=2, space="PSUM") as ps_wide,
                    ):
                        zB = tailp.tile([128, T], F32, name="zB")
                        scr = tailp.tile([128, T], F32, name="scr")
                        zcol = tailp.tile([128, 32], F32, name="zcol")
                        coltmp = tailp.tile([128, 16], F32, name="coltmp")
                        colw = tailp.tile([128, 64], F32, name="colw")
                        mask_u8 = tailp.tile([128, 32], mybir.dt.uint8, name="mask_u8")

                        def pe_bcast_col(src11, dst_col):
                            p = ps_small.tile([128, 1], F32, tag="bc")
                            nc.tensor.matmul(
                                p[:], lhsT=ones1x128[:], rhs=src11, start=True, stop=True
                            )
                            nc.vector.tensor_copy(dst_col, p[:])

                        for ch in range(NCH):
                            sl = slice(ch * CW, (ch + 1) * CW)
                            pb = ps_wide.tile([128, CW], F32, tag="pb")
                            nc.tensor.matmul(
                                pb[:], lhsT=ones1x128[:], rhs=z_row[0:1, sl],
                                start=True, stop=True,
                            )
                            if ch % 2 == 0:
                                nc.vector.tensor_copy(zB[:, sl], pb[:])
                            else:
                                nc.scalar.activation(zB[:, sl], pb[:], AF.Copy)
                        nc.sync.dma_start(zrow_d[:], z_row)
                        nc.sync.dma_start(
                            zcol[:], zrow_d[:].rearrange("o (p c) -> (o p) c", p=128)
                        )
                        mn_c = coltmp[:, 0:1]
                        mx_c = coltmp[:, 1:2]
                        nc.vector.tensor_reduce(
                            out=mn_c, in_=zcol[:], axis=mybir.AxisListType.X, op=ALU.min
                        )
                        nc.vector.reduce_max(
                            out=mx_c, in_=zcol[:], axis=mybir.AxisListType.X
                        )
                        ptr = ps_small.tile([1, 128], F32, tag="tr")
                        nc.tensor.transpose(ptr[:], mn_c, ident[:])
                        lo0 = strip[0:1, 0:1]
                        nc.vector.tensor_reduce(
                            out=lo0, in_=ptr[:], axis=mybir.AxisListType.X, op=ALU.min
                        )
                        ptr2 = ps_small.tile([1, 128], F32, tag="tr")
                        nc.tensor.transpose(ptr2[:], mx_c, ident[:])
                        hi0 = strip[0:1, 1:2]
                        nc.vector.reduce_max(
                            out=hi0, in_=ptr2[:], axis=mybir.AxisListType.X
                        )

                        N_ROUNDS = 3
                        lo_cur, hi_cur = lo0, hi0
                        si = 2
                        tau_col = coltmp[:, 2:3]
                        dB = coltmp[:, 3:4]
                        loB = coltmp[:, 4:5]
                        cnt_col = coltmp[:, 5:6]
                        sgn_col = coltmp[:, 6:7]
                        for r in range(N_ROUNDS):
                            d0 = strip[0:1, si : si + 1]
                            nc.vector.tensor_sub(d0, hi_cur, lo_cur)
                            pe_bcast_col(d0, dB)
                            pe_bcast_col(lo_cur, loB)
                            nc.vector.tensor_mul(tau_col, alphac[:], dB)
                            nc.vector.tensor_add(tau_col, tau_col, loB)
                            nc.vector.scalar_tensor_tensor(
                                out=scr[:],
                                in0=zB[:],
                                scalar=tau_col,
                                in1=zB[:],
                                op0=ALU.is_gt,
                                op1=ALU.bypass,
                                accum_out=cnt_col,
                            )
                            nc.vector.tensor_scalar(
                                out=sgn_col, in0=cnt_col, scalar1=float(SLOTS),
                                scalar2=None, op0=ALU.is_ge,
                            )
                            pj = ps_small.tile([1, 1], F32, tag="pj")
                            nc.tensor.matmul(
                                pj[:], lhsT=sgn_col, rhs=onescol[:], start=True, stop=True
                            )
                            dd = strip[0:1, si + 1 : si + 2]
                            nc.vector.tensor_scalar(
                                out=dd, in0=d0, scalar1=1.0 / 128, scalar2=None,
                                op0=ALU.mult,
                            )
                            tmp = strip[0:1, si + 2 : si + 3]
                            nc.vector.tensor_mul(tmp, pj[:], dd)
                            lo_n = strip[0:1, si + 3 : si + 4]
                            nc.vector.tensor_add(lo_n, lo_cur, tmp)
                            hi_n = strip[0:1, si + 4 : si + 5]
                            nc.vector.tensor_add(hi_n, lo_n, dd)
                            lo_cur, hi_cur = lo_n, hi_n
                            si += 5

                        # v = min(z > lo_cur), exactly
                        loB2 = coltmp[:, 8:9]
                        pe_bcast_col(lo_cur, loB2)
                        nc.vector.tensor_scalar(
                            out=mask_u8[:], in0=zcol[:], scalar1=loB2, scalar2=None,
                            op0=ALU.is_gt,
                        )
                        w_c = colw[:, 32:64]
                        nc.vector.memset(w_c, BIG)
                        nc.vector.copy_predicated(w_c, mask_u8[:], zcol[:])
                        wmin_c = coltmp[:, 9:10]
                        nc.vector.tensor_reduce(
                            out=wmin_c, in_=w_c, axis=mybir.AxisListType.X, op=ALU.min
                        )
                        ptr3 = ps_small.tile([1, 128], F32, tag="tr")
                        nc.tensor.transpose(ptr3[:], wmin_c, ident[:])
                        v0 = strip[0:1, si : si + 1]
                        nc.vector.tensor_reduce(
                            out=v0, in_=ptr3[:], axis=mybir.AxisListType.X, op=ALU.min
                        )
                        # c2 = count(z > v), need = 256 - c2
                        vB = coltmp[:, 10:11]
                        pe_bcast_col(v0, vB)
                        gt_c = colw[:, 0:32]
                        c2p = coltmp[:, 11:12]
                        nc.vector.scalar_tensor_tensor(
                            out=gt_c, in0=zcol[:], scalar=vB, in1=zcol[:],
                            op0=ALU.is_gt, op1=ALU.bypass, accum_out=c2p,
                        )
                        pc2 = ps_small.tile([1, 1], F32, tag="pj")
                        nc.tensor.matmul(
                            pc2[:], lhsT=c2p, rhs=onescol[:], start=True, stop=True
                        )
                        need0 = strip[0:1, si + 1 : si + 2]
                        nc.vector.tensor_scalar(
                            out=need0, in0=pc2[:], scalar1=float(SLOTS), scalar2=-1.0,
                            op0=ALU.subtract, op1=ALU.mult,
                        )
                        # tie-fix (row space)
                        maskeq = s_cnt
                        nc.vector.tensor_scalar(
                            out=maskeq, in0=z_row, scalar1=v0, scalar2=None,
                            op0=ALU.is_equal,
                        )
                        incl = s_z
                        nc.vector.tensor_tensor_scan(
                            incl, maskeq, maskeq, 0.0, op0=ALU.add, op1=ALU.bypass
                        )
                        fill = s_cs
                        nc.vector.scalar_tensor_tensor(
                            out=fill, in0=incl, scalar=need0, in1=maskeq,
                            op0=ALU.is_le, op1=ALU.mult,
                        )
                        maskgtv = s_z
                        nc.vector.tensor_scalar(
                            out=maskgtv, in0=z_row, scalar1=v0, scalar2=None,
                            op0=ALU.is_gt,
                        )
                        sel_row = s_cnt
                        nc.vector.tensor_add(sel_row, maskgtv, fill)
                        nc.sync.dma_start(dbg[1:2, :], sel_row)

                        # masked softmax + ctx
                        selm1 = s_z
                        nc.vector.tensor_scalar(
                            out=selm1, in0=sel_row, scalar1=-1.0, scalar2=None,
                            op0=ALU.add,
                        )
                        masked = s_phi
                        nc.vector.scalar_tensor_tensor(
                            out=masked, in0=selm1, scalar=BIG, in1=s_sc,
                            op0=ALU.mult, op1=ALU.add,
                        )
                        smax = strip[0:1, si + 2 : si + 3]
                        nc.vector.reduce_max(
                            out=smax, in_=masked, axis=mybir.AxisListType.X
                        )
                        nsmax = strip[0:1, si + 3 : si + 4]
                        nc.vector.tensor_scalar(
                            out=nsmax, in0=smax, scalar1=-1.0, scalar2=None, op0=ALU.mult
                        )
                        e_row = s_cs
                        zsum = strip[0:1, si + 4 : si + 5]
                        nc.scalar.activation(
                            e_row, masked, AF.Exp, bias=nsmax, accum_out=zsum
                        )
                        nc.sync.dma_start(dbg[3:4, :], e_row)
                        nc.sync.dma_start(dbg[4:5, :], sel_row)
                        rz = strip[0:1, si + 5 : si + 6]
                        nc.vector.reciprocal(out=rz, in_=zsum)
                        for ch in range(NCH):
                            sl = slice(ch * CW, (ch + 1) * CW)
                            pb = ps_wide.tile([128, CW], F32, tag="pb")
                            nc.tensor.matmul(
                                pb[:], lhsT=ones1x128[:], rhs=e_row[0:1, sl],
                                start=True, stop=True,
                            )
                            nc.vector.tensor_copy(zB[:, sl], pb[:])
                        for k in range(NK):
                            nc.vector.scalar_tensor_tensor(
                                out=scr[:],
                                in0=hT[k][:].bitcast(F32),
                                scalar=1.0,
                                in1=zB[:],
                                op0=ALU.mult,
                                op1=ALU.mult,
                                accum_out=ctx4[:, k : k + 1],
                            )
                        rzB = coltmp[:, 13:14]
                        pe_bcast_col(rz, rzB)
                        nc.vector.tensor_scalar(
                            out=ctx4[:], in0=ctx4[:], scalar1=rzB, scalar2=None,
                            op0=ALU.mult,
                        )
                        if DEBUG_HT:
                            for k in range(NK):
                                nc.sync.dma_start(
                                    htdump[128 * k : 128 * (k + 1), :],
                                    hT[k][:].bitcast(F32),
                                )
                # hT/arena/tail pools closed: SBUF free for full out_W prefetch

            # ---------------- allgather + output projection ----------------
            nc.sync.dma_start(cc_in[:], ctx4[:])
            nc.gpsimd.collective_compute(
                "AllGather",
                ALU.bypass,
                replica_groups=[list(range(NCORES))],
                ins=[cc_in[:]],
                outs=[cc_out[:]],
            )
            nc.sync.dma_start(
                ctxall[:].rearrange("p (j b) -> p j b", j=NK),
                cc_out[:].rearrange("(b p) j -> p j b", p=128),
            )
            ctxall_h = cpool.tile([128, 32], FDT, name="ctxall_h")
            nc.vector.tensor_copy(ctxall_h[:], ctxall[:])
            nchunks = (VS + CW - 1) // CW
            with (
                tc.tile_pool(name="wo", bufs=nchunks) as wopool,
                tc.tile_pool(name="bo", bufs=nchunks) as bopool,
                tc.tile_pool(name="lo", bufs=4) as lopool,
                tc.tile_pool(name="psl", bufs=4, space="PSUM") as psl,
            ):
                for n in range(nchunks):
                    w = min(CW, VS - n * CW)
                    vsl = slice(n * CW, n * CW + w)
                    wts = []
                    for k in range(NK):
                        wt = wopool.tile([128, CW], FDT, name=f"wo{k}", tag=f"wo{k}")
                        nc.sync.dma_start(wt[:, :w], wout[128 * k : 128 * (k + 1), vsl])
                        wts.append(wt)
                    bt = bopool.tile([1, CW], F32, name="bo", tag="bo")
                    nc.sync.dma_start(bt[:, :w], bout[:, vsl])
                    pl = psl.tile([B, CW], F32, tag="pl")
                    for k in range(NK):
                        nc.tensor.matmul(
                            pl[:, :w],
                            lhsT=ctxall_h[:, 8 * k : 8 * (k + 1)],
                            rhs=wts[k][:, :w],
                            start=(k == 0),
                            stop=False,
                        )
                    nc.tensor.matmul(
                        pl[:, :w], lhsT=ones1x8[:], rhs=bt[:, :w], start=False, stop=True
                    )
                    lt = lopool.tile([B, CW], F32, name="lt", tag="lt")
                    nc.vector.tensor_copy(lt[:, :w], pl[:, :w])
                    nc.sync.dma_start(logits[:, vsl], lt[:, :w])

    return nc


def _host_prep(inputs):
    f32 = lambda a: np.ascontiguousarray(np.asarray(a, dtype=np.float32))
    seq = np.asarray(inputs["seq"])
    embed = f32(inputs["embed"])
    w1 = f32(inputs["W1"])
    b1 = f32(inputs["b1"])
    w2 = f32(inputs["W2"])
    b2 = f32(inputs["b2"])
    ln_g = f32(inputs["ln_g"])
    ln_b = f32(inputs["ln_b"])
    gw = f32(inputs["gate_W"])
    gb = f32(inputs["gate_b"])
    qw = f32(inputs["q_W"])
    qb = f32(inputs["q_b"])
    wout = f32(inputs["out_W"])
    bout = f32(inputs["out_b"])

    colpack = lambda v: np.ascontiguousarray(
        v.reshape(-1, 128).T.astype(np.float32)
    )  # (Ntiles*128,) -> (128, Ntiles); tile k col = dims [128k, 128k+128)
    cnt = np.minimum(K, T - 1 - np.arange(T)).astype(np.float32)
    cntrec = np.zeros(T, dtype=np.float32)
    cntrec[cnt > 0] = 1.0 / cnt[cnt > 0]

    base = {
        "embed": embed,
        "w1": w1,
        "w2": w2,
        "qw": qw,
        "b1c": colpack(b1),
        "b2c": colpack(b2),
        "lngr": np.ascontiguousarray(ln_g.reshape(1, D)),
        "lnbc": colpack(ln_b),
        "gwc": np.concatenate([colpack(gw[:D, 0]), colpack(gw[D:, 0])], axis=1),
        "qbc": colpack(qb),
        "ident": np.eye(128, dtype=np.float32),
        "onesc": np.ones((128, 1), dtype=np.float32),
        "ones1x128": np.ones((1, 128), dtype=np.float32),
        "ones1x8": np.ones((1, 8), dtype=np.float32),
        "alphac": ((np.arange(128, dtype=np.float32) + 1.0) / 128.0).reshape(128, 1),
        "cntrec": cntrec.reshape(1, T),
        "sc_in": np.array([[EPS, float(gb[0]), 0.0, 0.0]], dtype=np.float32),
    }
    wout_pad = np.zeros((D, NCORES * VS), dtype=np.float32)
    wout_pad[:, :V] = wout
    bout_pad = np.zeros(NCORES * VS, dtype=np.float32)
    bout_pad[:V] = bout

    in_maps = []
    for c in range(NCORES):
        m = dict(base)
        m["seq_idx"] = np.ascontiguousarray(
            seq[c].reshape(32, 128).T.astype(np.int32)
        )
        fdt = np.float32 if PURE_FP32 else np.float16
        m["wout"] = np.ascontiguousarray(wout_pad[:, c * VS : (c + 1) * VS].astype(fdt))
        m["bout"] = np.ascontiguousarray(bout_pad[c * VS : (c + 1) * VS].reshape(1, VS))
        in_maps.append(m)
    return in_maps


def get_nc():
    key = (PURE_FP32, DEBUG_HT)
    if key not in _cache:
        _cache[key] = build_bass()
    return _cache[key]


def run_full(inputs, trace=False):
    """Run the kernel; returns (logits_full, raw_results, BassKernelResults)."""
    nc = get_nc()
    in_maps = _host_prep(inputs)
    res = run_bass_kernel_spmd(
        nc, in_maps, core_ids=list(range(NCORES)), trace=trace
    )
    parts = [res.results[c]["logits"] for c in range(NCORES)]
    logits = np.concatenate(parts, axis=1)[:, :V]
    return logits, res


def kernel(**inputs) -> np.ndarray:
    logits, _ = run_full(inputs, trace=False)
    return logits

